# revision 35
# baseline (speedup 1.0000x reference)
"""Trainium2 Bass kernel for nn_DigitConvolutionalModel (dense_cnn).

Model: y = relu(conv3x3(x) @ w1.T + b1) @ w2.T + b2, x: [65536, 784] f32.

Strategy:
  * The 3x3 valid conv (784 -> 676) and FC1 (676 -> 128) are both linear,
    so they fuse on the host into one effective weight W1e = w1 @ C with
    shape [128, 784] (C is the sparse conv operator). The device then runs
    a pure GEMM pipeline: y = relu(x @ W1e.T + b1) @ w2.T + b2.
  * Pure data parallel over 8 NeuronCores: each core gets 8192 rows of x.
  * Per-core shards are pre-transposed on the host to xT [784, 8192] so the
    contraction dim lands on SBUF partitions with fully contiguous DMA loads
    (no on-chip transposes; DMA x-bar transpose is 2-byte-dtype only).
  * On device, per 512-column batch block: 7 accumulating matmuls
    (K=112 each) into PSUM [128, 512], fused bias+ReLU on the scalar engine,
    one matmul [10, 512] for FC2, bias add on the vector engine, store.
    Output comes back as yT [10, 8192] per core; host transposes/concats.
"""

import os

import numpy as np

import concourse.bass as bass
import concourse.mybir as mybir
import concourse.tile as tile
from concourse.bass import ts
from concourse.bass_utils import run_bass_kernel_spmd

H = W = 28
KH = KW = 3
CIN = H * W  # 784
HID = 128
OUT = 10
B_TOTAL = 65536
NCORES = 8
BS = B_TOTAL // NCORES  # 8192 rows per core
NB = 512  # batch columns per psum block (fp32 PSUM bank limit)
NBLK = BS // NB  # 16
NLOAD = 1024  # batch columns per x DMA (~3.2 MB transfers)
NSUB = NLOAD // NB  # psum blocks per load
# contraction split: 6 full-partition chunks of 128 (keeps all 16 SDMA
# engines loaded on the big x DMAs) + a 16-row tail chunk
KCH = 128
KC = 6  # full chunks (6 * 128 = 768)
KTAIL = CIN - KC * KCH  # 16

# Matmul operand dtype. fp16 (e5m10): tf32-class accuracy for this model's
# value ranges (|x|<6, |h|<13), 1 cycle/row on the PE with fast weight
# load, and half the HBM bytes for x. "f32r" = single-pass reduced fp32
# (same accuracy class, but 4-byte DMA traffic); "f32" = exact.
MM_MODE = os.environ.get("BASS_MM_DT", "f16")
if os.environ.get("BASS_FP32R") == "0":  # legacy switch used by simcheck
    MM_MODE = "f32"
HOST_DT = np.float16 if MM_MODE == "f16" else np.float32


def _build_nc():
    f32 = mybir.dt.float32
    # Matmul-operand dtype: float32r tells the PE to run its single-pass
    # reduced-precision fp32 mode. The BIR verifier requires fp32r matmul
    # inputs to be *typed* fp32r at their producer, so the DRAM tensors and
    # SBUF tiles feeding matmuls carry this dtype (numpy side is still f32;
    # same 4-byte layout).
    mdt = {
        "f16": mybir.dt.float16,
        "f32r": mybir.dt.float32r,
        "f32": f32,
    }[MM_MODE]
    nc = bass.Bass()
    # x big part, host-pretiled to [load, partition, chunk, col] so each
    # per-load DMA is one contiguous region (128 descriptors x 24 KB)
    xb = nc.dram_tensor(
        "xb", [BS // NLOAD, KCH, KC, NLOAD], mdt, kind="ExternalInput"
    )
    # x contraction tail (features 768..783) for the whole batch
    xtl = nc.dram_tensor("xtl", [KTAIL, BS], mdt, kind="ExternalInput")
    w1t = nc.dram_tensor("w1t", [CIN, HID], mdt, kind="ExternalInput")
    b1d = nc.dram_tensor("b1d", [HID, 1], f32, kind="ExternalInput")
    w2t = nc.dram_tensor("w2t", [HID, OUT], mdt, kind="ExternalInput")
    b2d = nc.dram_tensor("b2d", [OUT, 1], f32, kind="ExternalInput")
    yt = nc.dram_tensor("yt", [OUT, BS], f32, kind="ExternalOutput")

    with tile.TileContext(nc) as tc:
        with (
            tc.tile_pool(name="consts", bufs=1) as consts,
            tc.tile_pool(name="xin", bufs=4) as xin,
            tc.tile_pool(name="hpool", bufs=3) as hpool,
            tc.tile_pool(name="opool", bufs=3) as opool,
            tc.tile_pool(name="ps1", bufs=2, space="PSUM") as ps1p,
            tc.tile_pool(name="ps2", bufs=2, space="PSUM") as ps2p,
        ):
            # Issue the first big x load before anything else so the main
            # DMA stream starts as early as possible (weights are tiny and
            # only gate the PE, which has plenty of slack).
            x_t0 = xin.tile([KCH, KC, NLOAD], mdt, tag="x_t")
            nc.sync.dma_start(x_t0[:], xb[0])

            # FC1 weight, chunked [k, chunk, hid]: partition k in 0..127,
            # chunk c selects rows c*128..c*128+127 of w1t; plus 16-row tail.
            w1_t = consts.tile([KCH, KC, HID], mdt)
            nc.sync.dma_start(
                w1_t[:], w1t[0 : KC * KCH, :].rearrange("(c k) m -> k c m", k=KCH)
            )
            w1_tail = consts.tile([KTAIL, HID], mdt)
            nc.sync.dma_start(w1_tail[:], w1t[KC * KCH :, :])
            b1_t = consts.tile([HID, 1], f32)
            nc.sync.dma_start(b1_t[:], b1d[:])
            w2_t = consts.tile([HID, OUT], mdt)
            nc.sync.dma_start(w2_t[:], w2t[:])
            b2_t = consts.tile([OUT, 1], f32)
            nc.sync.dma_start(b2_t[:], b2d[:])

            # Pre-touch the bias tiles on their consumer engine (DVE) so the
            # main relu / bias-add instructions don't need a second sync-wait
            # for the bias DMA (walrus allows 1 wait per inst).
            b1_probe = consts.tile([1, 1], f32)
            nc.vector.tensor_copy(b1_probe[:], b1_t[0:1, 0:1])
            b2_probe = consts.tile([1, 1], f32)
            nc.vector.tensor_copy(b2_probe[:], b2_t[0:1, 0:1])

            # fp32/fp32r matmuls self-load their weights (no separate
            # LDWEIGHTS), so every semaphore wait lands on the Matmult
            # itself — and walrus only allows one sync-wait there. Tiny
            # dummy bf16 ldweights "probes" reading 1 element of a tile
            # absorb the cross-engine waits into the PE's in-order stream
            # before each matmul group. The loaded garbage weight is
            # irrelevant (the real matmuls self-load).
            def probe(ap):
                nc.tensor.ldweights(ap[0:1, 0:1].bitcast(mybir.dt.bfloat16))

            # the 16-row contraction tail for the whole batch, loaded once
            x_tl = consts.tile([KTAIL, BS], mdt)
            nc.sync.dma_start(x_tl[:], xtl[:])

            probe(w1_t[:, 0, :])
            probe(w1_tail[:])
            probe(x_tl[:])
            probe(w2_t[:])

            def fc2_and_store(h, li):
                # h(li) finished while the next load's FC1 ran, so these
                # matmuls never stall the PE stream.
                probe(h[:, 0, :])
                ps2 = ps2p.tile([OUT, NSUB, NB], f32)
                for si in range(NSUB):
                    nc.tensor.matmul(
                        ps2[:, si, :],
                        w2_t[:],
                        h[:, si, :],
                        start=True,
                        stop=True,
                    )
                o = opool.tile([OUT, NSUB, NB], f32)
                nc.vector.tensor_scalar_add(o[:], ps2[:], b2_t[:])
                nc.scalar.dma_start(yt[:, ts(li, NLOAD)], o[:])

            def relu_block(ps, li):
                # fused bias + relu on DVE over both banks: h = max(ps+b1, 0)
                h = hpool.tile([HID, NSUB, NB], mdt)
                nc.vector.tensor_scalar(
                    h[:],
                    ps[:],
                    b1_t[:],
                    0.0,
                    mybir.AluOpType.add,
                    mybir.AluOpType.max,
                )
                return h

            # Software pipeline, 2 stages deep: iteration li runs FC1(li) on
            # the PE, relu(li-1) on the DVE, and FC2(li-2) on the PE. Every
            # cross-engine dependency (even when Tile coarsens a semaphore
            # wait to a later producer) was satisfied a full load earlier,
            # so neither engine stalls.
            ps_pend = None  # (ps, li)
            fc2_pend = []  # (h, li)
            for li in range(BS // NLOAD):
                if li == 0:
                    x_t = x_t0
                else:
                    x_t = xin.tile([KCH, KC, NLOAD], mdt, tag="x_t")
                    nc.sync.dma_start(x_t[:], xb[li])

                probe(x_t[:, 0, :])
                # c-outer over both psum banks: same stationary weight for
                # two back-to-back matmuls
                ps = ps1p.tile([HID, NSUB, NB], f32)
                for c in range(KC):
                    for si in range(NSUB):
                        nc.tensor.matmul(
                            ps[:, si, :],
                            w1_t[:, c, :],
                            x_t[:, c, ts(si, NB)],
                            start=(c == 0),
                            stop=False,
                        )
                for si in range(NSUB):
                    nc.tensor.matmul(
                        ps[:, si, :],
                        w1_tail[:],
                        x_tl[:, ts(li * NSUB + si, NB)],
                        start=False,
                        stop=True,
                    )

                if ps_pend is not None:
                    fc2_pend.append((relu_block(*ps_pend), ps_pend[1]))
                if len(fc2_pend) == 2:
                    fc2_and_store(*fc2_pend.pop(0))
                ps_pend = (ps, li)

            fc2_pend.append((relu_block(*ps_pend), ps_pend[1]))
            for p in fc2_pend:
                fc2_and_store(*p)

    # This walrus build allows one sync-wait per instruction; Tile emits
    # multi-waits (e.g. slot-recycle WAW + readers-release on DMAs). Split
    # them into event-semaphore chains, same as bacc.compile() does.
    import bass_rust

    bass_rust.generate_event_semaphores(nc)
    return nc


def _fuse_conv_fc1(conv_w, w1):
    """W1e = w1 @ C where C is the 3x3 valid-conv operator [676, 784]."""
    cw = np.asarray(conv_w, np.float64).reshape(KH, KW)
    w1_r = np.asarray(w1, np.float64).reshape(HID, H - KH + 1, W - KW + 1)
    w1e = np.zeros((HID, H, W), np.float64)
    for a in range(KH):
        for b in range(KW):
            w1e[:, a : a + H - KH + 1, b : b + W - KW + 1] += w1_r * cw[a, b]
    return w1e.reshape(HID, CIN).astype(np.float32)


def _core_x(x_shard):
    """Pre-tile one core's x rows [BS, 784] into the device layout:
    xb [nload, k, c, n] (features 0..767) and xtl [16, BS] (tail)."""
    xb = np.ascontiguousarray(
        x_shard[:, : KC * KCH]
        .reshape(BS // NLOAD, NLOAD, KC, KCH)
        .transpose(0, 3, 2, 1)
        .astype(HOST_DT)
    )
    xtl = np.ascontiguousarray(x_shard[:, KC * KCH :].T.astype(HOST_DT))
    return xb, xtl


def _run(x, conv_w, w1, b1, w2, b2, trace=False):
    x = np.asarray(x, np.float32)
    w1e_t = np.ascontiguousarray(_fuse_conv_fc1(conv_w, w1).T.astype(HOST_DT))
    w2t = np.ascontiguousarray(np.asarray(w2, np.float32).T.astype(HOST_DT))
    b1c = np.ascontiguousarray(np.asarray(b1, np.float32).reshape(HID, 1))
    b2c = np.ascontiguousarray(np.asarray(b2, np.float32).reshape(OUT, 1))

    nc = _build_nc()
    in_maps = []
    for c in range(NCORES):
        xb, xtl = _core_x(x[c * BS : (c + 1) * BS])
        in_maps.append(
            {"xb": xb, "xtl": xtl, "w1t": w1e_t, "b1d": b1c, "w2t": w2t, "b2d": b2c}
        )
    res = run_bass_kernel_spmd(nc, in_maps, list(range(NCORES)), trace=trace)

    y = np.empty((B_TOTAL, OUT), np.float32)
    for c, r in enumerate(res.results):
        y[c * BS : (c + 1) * BS] = r["yt"].T
    return y, res


def kernel(x, conv_w, w1, b1, w2, b2):
    y, _ = _run(x, conv_w, w1, b1, w2, b2)
    return y


# revision 37
# speedup vs baseline: 1.2215x; 1.2215x over previous
"""Trainium2 Bass kernel for nn_DigitConvolutionalModel (dense_cnn).

Model: y = relu(conv3x3(x) @ w1.T + b1) @ w2.T + b2, x: [65536, 784] f32.

Strategy:
  * The 3x3 valid conv (784 -> 676) and FC1 (676 -> 128) are both linear,
    so they fuse on the host into one effective weight W1e = w1 @ C with
    shape [128, 784] (C is the sparse conv operator). The device then runs
    a pure GEMM pipeline: y = relu(x @ W1e.T + b1) @ w2.T + b2.
  * Pure data parallel over 8 NeuronCores: each core gets 8192 rows of x.
  * Per-core shards are pre-transposed on the host to xT [784, 8192] so the
    contraction dim lands on SBUF partitions with fully contiguous DMA loads
    (no on-chip transposes; DMA x-bar transpose is 2-byte-dtype only).
  * On device, per 512-column batch block: 7 accumulating matmuls
    (K=112 each) into PSUM [128, 512], fused bias+ReLU on the scalar engine,
    one matmul [10, 512] for FC2, bias add on the vector engine, store.
    Output comes back as yT [10, 8192] per core; host transposes/concats.
"""

import os

import numpy as np

import concourse.bass as bass
import concourse.mybir as mybir
import concourse.tile as tile
from concourse.bass import ts
from concourse.bass_utils import run_bass_kernel_spmd

H = W = 28
KH = KW = 3
CIN = H * W  # 784
HID = 128
OUT = 10
B_TOTAL = 65536
NCORES = 8
BS = B_TOTAL // NCORES  # 8192 rows per core
NB = 512  # batch columns per psum block (fp32 PSUM bank limit)
NBLK = BS // NB  # 16
NLOAD = 1024  # batch columns per x DMA (~3.2 MB transfers)
NSUB = NLOAD // NB  # psum blocks per load
# contraction split: 6 full-partition chunks of 128 (keeps all 16 SDMA
# engines loaded on the big x DMAs) + a 16-row tail chunk
KCH = 128
KC = 6  # full chunks (6 * 128 = 768)
KTAIL = CIN - KC * KCH  # 16

# Matmul operand dtype. fp16 (e5m10): tf32-class accuracy for this model's
# value ranges (|x|<6, |h|<13), 1 cycle/row on the PE with fast weight
# load, and half the HBM bytes for x. "f32r" = single-pass reduced fp32
# (same accuracy class, but 4-byte DMA traffic); "f32" = exact.
MM_MODE = os.environ.get("BASS_MM_DT", "f16")
if os.environ.get("BASS_FP32R") == "0":  # legacy switch used by simcheck
    MM_MODE = "f32"
HOST_DT = np.float16 if MM_MODE == "f16" else np.float32


def _build_nc():
    f32 = mybir.dt.float32
    # Matmul-operand dtype: float32r tells the PE to run its single-pass
    # reduced-precision fp32 mode. The BIR verifier requires fp32r matmul
    # inputs to be *typed* fp32r at their producer, so the DRAM tensors and
    # SBUF tiles feeding matmuls carry this dtype (numpy side is still f32;
    # same 4-byte layout).
    mdt = {
        "f16": mybir.dt.float16,
        "f32r": mybir.dt.float32r,
        "f32": f32,
    }[MM_MODE]
    nc = bass.Bass()
    # x big part, host-pretiled to [load, partition, chunk, col] so each
    # per-load DMA is one contiguous region (128 descriptors x 24 KB)
    xb = nc.dram_tensor(
        "xb", [BS // NLOAD, KCH, KC, NLOAD], mdt, kind="ExternalInput"
    )
    # x contraction tail (features 768..783) for the whole batch
    xtl = nc.dram_tensor("xtl", [KTAIL, BS], mdt, kind="ExternalInput")
    w1t = nc.dram_tensor("w1t", [CIN, HID], mdt, kind="ExternalInput")
    b1d = nc.dram_tensor("b1d", [HID, 1], f32, kind="ExternalInput")
    w2t = nc.dram_tensor("w2t", [HID, OUT], mdt, kind="ExternalInput")
    b2d = nc.dram_tensor("b2d", [OUT, 1], f32, kind="ExternalInput")
    yt = nc.dram_tensor("yt", [OUT, BS], f32, kind="ExternalOutput")

    with tile.TileContext(nc) as tc:
        with (
            tc.tile_pool(name="consts", bufs=1) as consts,
            tc.tile_pool(name="xin", bufs=4) as xin,
            tc.tile_pool(name="hpool", bufs=4) as hpool,
            tc.tile_pool(name="opool", bufs=3) as opool,
            tc.tile_pool(name="ps1", bufs=4, space="PSUM") as ps1p,
            tc.tile_pool(name="ps2", bufs=2, space="PSUM") as ps2p,
        ):
            # Issue the first big x load before anything else so the main
            # DMA stream starts as early as possible (weights are tiny and
            # only gate the PE, which has plenty of slack).
            x_t0 = xin.tile([KCH, KC, NLOAD], mdt, tag="x_t")
            nc.sync.dma_start(x_t0[:], xb[0])

            # FC1 weight, chunked [k, chunk, hid]: partition k in 0..127,
            # chunk c selects rows c*128..c*128+127 of w1t; plus 16-row tail.
            w1_t = consts.tile([KCH, KC, HID], mdt)
            nc.sync.dma_start(
                w1_t[:], w1t[0 : KC * KCH, :].rearrange("(c k) m -> k c m", k=KCH)
            )
            w1_tail = consts.tile([KTAIL, HID], mdt)
            nc.sync.dma_start(w1_tail[:], w1t[KC * KCH :, :])
            b1_t = consts.tile([HID, 1], f32)
            nc.sync.dma_start(b1_t[:], b1d[:])
            w2_t = consts.tile([HID, OUT], mdt)
            nc.sync.dma_start(w2_t[:], w2t[:])
            b2_t = consts.tile([OUT, 1], f32)
            nc.sync.dma_start(b2_t[:], b2d[:])

            # Pre-touch the bias tiles on their consumer engine (DVE) so the
            # main relu / bias-add instructions don't need a second sync-wait
            # for the bias DMA (walrus allows 1 wait per inst).
            b1_probe = consts.tile([1, 1], f32)
            nc.vector.tensor_copy(b1_probe[:], b1_t[0:1, 0:1])
            b2_probe = consts.tile([1, 1], f32)
            nc.vector.tensor_copy(b2_probe[:], b2_t[0:1, 0:1])

            # fp32/fp32r matmuls self-load their weights (no separate
            # LDWEIGHTS), so every semaphore wait lands on the Matmult
            # itself — and walrus only allows one sync-wait there. Tiny
            # dummy bf16 ldweights "probes" reading 1 element of a tile
            # absorb the cross-engine waits into the PE's in-order stream
            # before each matmul group. The loaded garbage weight is
            # irrelevant (the real matmuls self-load).
            def probe(ap):
                nc.tensor.ldweights(ap[0:1, 0:1].bitcast(mybir.dt.bfloat16))

            # the 16-row contraction tail for the whole batch, loaded once
            x_tl = consts.tile([KTAIL, BS], mdt)
            nc.sync.dma_start(x_tl[:], xtl[:])

            probe(w1_t[:, 0, :])
            probe(w1_tail[:])
            probe(x_tl[:])
            probe(w2_t[:])

            # Per load: FC1 runs si-outer (bank si finishes early so its
            # relu overlaps the other bank's matmuls), then per-bank
            # relu -> FC2, then one bias-add + store for the whole load.
            for li in range(BS // NLOAD):
                if li == 0:
                    x_t = x_t0
                else:
                    x_t = xin.tile([KCH, KC, NLOAD], mdt, tag="x_t")
                    nc.sync.dma_start(x_t[:], xb[li])

                probe(x_t[:, 0, :])
                pss = []
                for si in range(NSUB):
                    ps = ps1p.tile([HID, NB], f32)
                    for c in range(KC):
                        nc.tensor.matmul(
                            ps[:],
                            w1_t[:, c, :],
                            x_t[:, c, ts(si, NB)],
                            start=(c == 0),
                            stop=False,
                        )
                    nc.tensor.matmul(
                        ps[:],
                        w1_tail[:],
                        x_tl[:, ts(li * NSUB + si, NB)],
                        start=False,
                        stop=True,
                    )
                    pss.append(ps)

                ps2 = ps2p.tile([OUT, NSUB, NB], f32)
                for si in range(NSUB):
                    # relu+bias on DVE: h = max(ps + b1, 0)
                    h = hpool.tile([HID, NB], mdt)
                    nc.vector.tensor_scalar(
                        h[:],
                        pss[si][:],
                        b1_t[:],
                        0.0,
                        mybir.AluOpType.add,
                        mybir.AluOpType.max,
                    )
                    probe(h[:])
                    nc.tensor.matmul(
                        ps2[:, si, :], w2_t[:], h[:], start=True, stop=True
                    )

                o = opool.tile([OUT, NSUB, NB], f32)
                nc.vector.tensor_scalar_add(o[:], ps2[:], b2_t[:])
                nc.scalar.dma_start(yt[:, ts(li, NLOAD)], o[:])

    # This walrus build allows one sync-wait per instruction; Tile emits
    # multi-waits (e.g. slot-recycle WAW + readers-release on DMAs). Split
    # them into event-semaphore chains, same as bacc.compile() does.
    import bass_rust

    bass_rust.generate_event_semaphores(nc)
    return nc


def _fuse_conv_fc1(conv_w, w1):
    """W1e = w1 @ C where C is the 3x3 valid-conv operator [676, 784]."""
    cw = np.asarray(conv_w, np.float64).reshape(KH, KW)
    w1_r = np.asarray(w1, np.float64).reshape(HID, H - KH + 1, W - KW + 1)
    w1e = np.zeros((HID, H, W), np.float64)
    for a in range(KH):
        for b in range(KW):
            w1e[:, a : a + H - KH + 1, b : b + W - KW + 1] += w1_r * cw[a, b]
    return w1e.reshape(HID, CIN).astype(np.float32)


def _core_x(x_shard):
    """Pre-tile one core's x rows [BS, 784] into the device layout:
    xb [nload, k, c, n] (features 0..767) and xtl [16, BS] (tail)."""
    xb = np.ascontiguousarray(
        x_shard[:, : KC * KCH]
        .reshape(BS // NLOAD, NLOAD, KC, KCH)
        .transpose(0, 3, 2, 1)
        .astype(HOST_DT)
    )
    xtl = np.ascontiguousarray(x_shard[:, KC * KCH :].T.astype(HOST_DT))
    return xb, xtl


def _run(x, conv_w, w1, b1, w2, b2, trace=False):
    x = np.asarray(x, np.float32)
    w1e_t = np.ascontiguousarray(_fuse_conv_fc1(conv_w, w1).T.astype(HOST_DT))
    w2t = np.ascontiguousarray(np.asarray(w2, np.float32).T.astype(HOST_DT))
    b1c = np.ascontiguousarray(np.asarray(b1, np.float32).reshape(HID, 1))
    b2c = np.ascontiguousarray(np.asarray(b2, np.float32).reshape(OUT, 1))

    nc = _build_nc()
    in_maps = []
    for c in range(NCORES):
        xb, xtl = _core_x(x[c * BS : (c + 1) * BS])
        in_maps.append(
            {"xb": xb, "xtl": xtl, "w1t": w1e_t, "b1d": b1c, "w2t": w2t, "b2d": b2c}
        )
    res = run_bass_kernel_spmd(nc, in_maps, list(range(NCORES)), trace=trace)

    y = np.empty((B_TOTAL, OUT), np.float32)
    for c, r in enumerate(res.results):
        y[c * BS : (c + 1) * BS] = r["yt"].T
    return y, res


def kernel(x, conv_w, w1, b1, w2, b2):
    y, _ = _run(x, conv_w, w1, b1, w2, b2)
    return y
